# revision 1
# baseline (speedup 1.0000x reference)
"""Bilateral filter (7x7, sigma_color=0.1) Trainium2 Bass kernel.

Full inputs: input_tensor [16, 3, 1024, 1024] fp32 in [0,1].
Sharding: batch-parallel — 48 channel-images split as 6 per core across 8 cores.

Per-core algorithm (one For_i loop over the 6 channel-images):
  1. Build an edge-padded copy [H+6, W+6] in a DRAM-pool tile via DMAs.
  2. For each 128-row band, DMA 7 row-shifted tiles T_dy [128, W+6] from the
     padded image (compute engines cannot shift partitions, DMA can).
  3. Per tap (dy,dx) != center:  d = T_dy[:, dx:dx+W] - center (DVE, bf16 out);
     q = d*d (ACT Square); w = exp(-50*q - ds2/18) (ACT, spatial weight folded
     into the bias); t = w*d (DVE bf16 2x); t and w accumulate through bf16
     binary trees with fp32 roots A and den (DVE).
  4. out = center + A * approx_reciprocal(1 + den); DMA band to output.
  Bands are processed in PAIRS (free dim 2*W) to amortize per-op overhead.
  GpSimd is intentionally idle: concurrent GpSimd elementwise traffic
  contends with the DVE for SBUF and slows every DVE op ~1.5x.
"""

import sys

sys.path.insert(0, "/opt/trn_rl_repo")

import numpy as np

SPATIAL_RADIUS = 3
COLOR_RADIUS = 0.1
INV_2C2 = 1.0 / (2.0 * COLOR_RADIUS**2)  # 50.0
INV_2R2 = 1.0 / (2.0 * float(SPATIAL_RADIUS) ** 2)  # 1/18

N_CORES = 8
_NC_CACHE = {}


def build_nc(n_img, H, W, variant="fp16A"):
    """Build the per-core Bass kernel: n_img channel-images of [H, W].

    variant:
      "fp32"  — all fp32: num/den accumulation of w and w*s.
      "bf16A" — A-formulation out = cen + (sum w*d)/(1 + sum w); d/q/w/t in
                bf16 (DVE 2x mode on the squares/products), A/den in fp32.
    """
    import concourse.bacc as bacc
    import concourse.bass as bass
    import concourse.mybir as mybir
    from concourse.tile import TileContext

    ds = bass.ds
    f32 = mybir.dt.float32
    bf16 = mybir.dt.bfloat16
    K = 2 * SPATIAL_RADIUS + 1  # 7
    R = SPATIAL_RADIUS  # 3
    Wp = W + 2 * R  # padded width
    P = 128  # band height (partitions)
    assert H % P == 0
    n_bands = H // P

    nc = bacc.Bacc(None, target_bir_lowering=False)
    x = nc.declare_dram_parameter("x", [n_img * H, W], f32, isOutput=False)
    y = nc.declare_dram_parameter("y", [n_img * H, W], f32, isOutput=True)

    # distinct spatial-weight classes: ds2 = (dy-R)^2 + (dx-R)^2
    ds2_vals = sorted({(dy - R) ** 2 + (dx - R) ** 2 for dy in range(K) for dx in range(K)} - {0})
    ds2_col = {v: i for i, v in enumerate(ds2_vals)}

    with TileContext(nc) as tc:
        with (
            tc.tile_pool(name="consts", bufs=1) as cpool,
            tc.tile_pool(name="drampool", bufs=2, space="DRAM") as dpool,
            tc.tile_pool(name="bandpool", bufs=2) as bpool,
            tc.tile_pool(name="workpool", bufs=2) as wpool,
            tc.tile_pool(name="accpool", bufs=1 if variant == "fp16A" else 2) as apool,
        ):
            bias = cpool.tile([P, len(ds2_vals)], f32)
            for v, i in ds2_col.items():
                nc.gpsimd.memset(bias[:, i : i + 1], -float(v) * INV_2R2)

            with tc.For_i(0, n_img * H, H) as gbase:
                pad = dpool.tile([H + 2 * R, Wp], f32, tag="pad")
                # ---- phase 0: build padded image in DRAM ----
                nc.sync.dma_start(out=pad[R : H + R, R : W + R], in_=x[ds(gbase, H), :])
                with nc.allow_non_contiguous_dma(reason="tiny edge-column pads"):
                    for ccol in range(R):
                        nc.sync.dma_start(out=pad[R : H + R, ccol : ccol + 1], in_=x[ds(gbase, H), 0:1])
                        nc.sync.dma_start(
                            out=pad[R : H + R, W + R + ccol : W + R + ccol + 1],
                            in_=x[ds(gbase, H), W - 1 : W],
                        )
                for rrow in range(R):
                    nc.sync.dma_start(out=pad[rrow : rrow + 1, :], in_=pad[R : R + 1, :])
                    nc.sync.dma_start(
                        out=pad[H + R + rrow : H + R + rrow + 1, :],
                        in_=pad[H + R - 1 : H + R, :],
                    )

                # ---- phase 1: bands ----
                if variant == "fp16A":
                    # fp16 path: T tiles cast to fp16 in TWO alignment parities so
                    # every sub hits the DVE 2x mode (fp16 slices at odd dx would
                    # be 2B-misaligned; the odd-parity copy shifts by one elem).
                    f16 = mybir.dt.float16
                    assert n_bands % 2 == 0
                    for pb in range(n_bands // 2):
                        r0 = pb * 2 * P
                        Tqmap = {}

                        def load_fp32(dy, tag, r0=r0):
                            t_dy = bpool.tile([P, 2 * Wp], f32, tag=tag)
                            nc.sync.dma_start(out=t_dy[:, 0:Wp], in_=pad[r0 + dy : r0 + dy + P, :])
                            nc.sync.dma_start(out=t_dy[:, Wp : 2 * Wp], in_=pad[r0 + P + dy : r0 + P + dy + P, :])
                            return t_dy

                        cen32 = load_fp32(R, "Tcen")

                        def get_Tq(dy, r0=r0, Tqmap=Tqmap):
                            if dy not in Tqmap:
                                if dy == R:
                                    src, base = cen32, "Tqc"
                                else:
                                    src, base = load_fp32(dy, f"Ts{dy % 2}"), f"Tq{dy % 2}"
                                te = bpool.tile([P, 2 * Wp], f16, tag=base + "e")
                                to = bpool.tile([P, 2 * Wp], f16, tag=base + "o")
                                nc.scalar.copy(te[:, :], src[:, :])
                                nc.scalar.copy(to[:, 0 : 2 * Wp - 2], src[:, 1 : 2 * Wp - 1])
                                Tqmap[dy] = (te, to)
                            return Tqmap[dy]

                        def seg16(pair, dx):
                            te, to = pair
                            if dx % 2 == 0:
                                base, off = te, dx
                            else:
                                base, off = to, dx - 1
                            return base[:, :].rearrange("p (s c) -> p s c", c=Wp)[:, :, off : off + W]

                        cen = seg16(get_Tq(R), R)
                        cen32v = cen32[:, :].rearrange("p (s c) -> p s c", c=Wp)[:, :, R : R + W]

                        acc = apool.tile([P, 2 * W], f32, tag="acc")
                        den = apool.tile([P, 2 * W], f32, tag="den")
                        acc3 = acc[:, :].rearrange("p (s c) -> p s c", c=W)
                        nc.vector.memset(acc[:, :], 0.0)
                        nc.vector.memset(den[:, :], 1.0)

                        taps = [(dy, dx) for dy in range(K) for dx in range(K) if not (dy == R and dx == R)]
                        GRP = 8

                        def fold_push(stack, tile, eng):
                            lv = 0
                            while stack and stack[-1][0] == lv:
                                _, prev = stack.pop()
                                eng.tensor_tensor(out=prev[:, :], in0=prev[:, :], in1=tile[:, :], op=mybir.AluOpType.add)
                                tile = prev
                                lv += 1
                            stack.append((lv, tile))

                        for g0 in range(0, len(taps), GRP):
                            group = taps[g0 : g0 + GRP]
                            tstack, wstack = [], []
                            for gi, (dy, dx) in enumerate(group):
                                s = seg16(get_Tq(dy), dx)
                                d = wpool.tile([P, 2 * W], f16, tag=f"d{gi % 2}")
                                q = wpool.tile([P, 2 * W], f16, tag="q")
                                w = wpool.tile([P, 2 * W], f16, tag=f"w{gi % 3}")
                                t = wpool.tile([P, 2 * W], f16, tag=f"t{gi % 3}")
                                d3 = d[:, :].rearrange("p (s c) -> p s c", c=W)
                                nc.vector.tensor_tensor(out=d3, in0=s, in1=cen, op=mybir.AluOpType.subtract)
                                nc.scalar.activation(q[:, :], d[:, :], mybir.ActivationFunctionType.Square)
                                ds2 = (dy - R) ** 2 + (dx - R) ** 2
                                nc.scalar.activation(
                                    w[:, :],
                                    q[:, :],
                                    mybir.ActivationFunctionType.Exp,
                                    bias=bias[:, ds2_col[ds2] : ds2_col[ds2] + 1],
                                    scale=-INV_2C2,
                                )
                                nc.vector.tensor_tensor(out=t[:, :], in0=w[:, :], in1=d[:, :], op=mybir.AluOpType.mult)
                                fold_push(tstack, t, nc.vector)
                                fold_push(wstack, w, nc.vector)
                            for stack, accum, eng in ((tstack, acc, nc.vector), (wstack, den, nc.vector)):
                                while len(stack) > 1:
                                    _, b2 = stack.pop()
                                    _, a2 = stack.pop()
                                    eng.tensor_tensor(out=a2[:, :], in0=a2[:, :], in1=b2[:, :], op=mybir.AluOpType.add)
                                    stack.append((99, a2))
                                eng.tensor_tensor(out=accum[:, :], in0=accum[:, :], in1=stack[0][1][:, :], op=mybir.AluOpType.add)

                        rcp = wpool.tile([P, 2 * W], f32, tag="w0")
                        scr = wpool.tile([P, 2 * W], f32, tag="w1")
                        nc.vector.reciprocal_approx_accurate(rcp[:, :], den[:, :], scr[:, :])
                        nc.vector.tensor_tensor(out=acc[:, :], in0=acc[:, :], in1=rcp[:, :], op=mybir.AluOpType.mult)
                        nc.vector.tensor_tensor(out=acc3, in0=acc3, in1=cen32v, op=mybir.AluOpType.add)
                        nc.sync.dma_start(out=y[ds(gbase + r0, P), :], in_=acc[:, 0:W])
                        nc.sync.dma_start(out=y[ds(gbase + r0 + P, P), :], in_=acc[:, W : 2 * W])
                elif variant == "bf16A":
                    # process PAIRS of 128-row bands side by side (free dim 2*W)
                    # to amortize per-instruction overhead.
                    assert n_bands % 2 == 0
                    for pb in range(n_bands // 2):
                        r0 = pb * 2 * P
                        Tmap = {}

                        def get_T(dy, r0=r0, Tmap=Tmap):
                            if dy not in Tmap:
                                tag = "Tcen" if dy == R else f"T{dy % 3}"
                                t_dy = bpool.tile([P, 2 * Wp], f32, tag=tag)
                                nc.sync.dma_start(out=t_dy[:, 0:Wp], in_=pad[r0 + dy : r0 + dy + P, :])
                                nc.sync.dma_start(out=t_dy[:, Wp : 2 * Wp], in_=pad[r0 + P + dy : r0 + P + dy + P, :])
                                Tmap[dy] = t_dy
                            return Tmap[dy]

                        def seg(tile_, dx):
                            return tile_[:, :].rearrange("p (s c) -> p s c", c=Wp)[:, :, dx : dx + W]

                        cen = seg(get_T(R), R)

                        acc = apool.tile([P, 2 * W], f32, tag="acc")
                        den = apool.tile([P, 2 * W], f32, tag="den")
                        acc3 = acc[:, :].rearrange("p (s c) -> p s c", c=W)
                        nc.vector.memset(acc[:, :], 0.0)
                        nc.gpsimd.memset(den[:, :], 1.0)

                        taps = [(dy, dx) for dy in range(K) for dx in range(K) if not (dy == R and dx == R)]
                        GRP = 8
                        sub_flip = 0

                        def fold_push(stack, tile, eng):
                            lv = 0
                            while stack and stack[-1][0] == lv:
                                _, prev = stack.pop()
                                eng.tensor_tensor(out=prev[:, :], in0=prev[:, :], in1=tile[:, :], op=mybir.AluOpType.add)
                                tile = prev
                                lv += 1
                            stack.append((lv, tile))

                        for g0 in range(0, len(taps), GRP):
                            group = taps[g0 : g0 + GRP]
                            tstack, wstack = [], []
                            for gi, (dy, dx) in enumerate(group):
                                s = seg(get_T(dy), dx)
                                d = wpool.tile([P, 2 * W], bf16, tag=f"d{gi % 2}")
                                q = wpool.tile([P, 2 * W], bf16, tag="q")
                                w = wpool.tile([P, 2 * W], bf16, tag=f"w{gi % 4}")
                                t = wpool.tile([P, 2 * W], bf16, tag=f"t{gi % 4}")
                                d3 = d[:, :].rearrange("p (s c) -> p s c", c=W)
                                sub_flip += 1
                                nc.vector.tensor_tensor(out=d3, in0=s, in1=cen, op=mybir.AluOpType.subtract)
                                # square on the (otherwise idle) scalar engine
                                nc.scalar.activation(q[:, :], d[:, :], mybir.ActivationFunctionType.Square)
                                ds2 = (dy - R) ** 2 + (dx - R) ** 2
                                nc.scalar.activation(
                                    w[:, :],
                                    q[:, :],
                                    mybir.ActivationFunctionType.Exp,
                                    bias=bias[:, ds2_col[ds2] : ds2_col[ds2] + 1],
                                    scale=-INV_2C2,
                                )
                                nc.vector.tensor_tensor(out=t[:, :], in0=w[:, :], in1=d[:, :], op=mybir.AluOpType.mult)
                                fold_push(tstack, t, nc.vector)
                                fold_push(wstack, w, nc.vector)
                            for stack, accum, eng in ((tstack, acc, nc.vector), (wstack, den, nc.vector)):
                                while len(stack) > 1:
                                    _, b2 = stack.pop()
                                    _, a2 = stack.pop()
                                    eng.tensor_tensor(out=a2[:, :], in0=a2[:, :], in1=b2[:, :], op=mybir.AluOpType.add)
                                    stack.append((99, a2))
                                eng.tensor_tensor(out=accum[:, :], in0=accum[:, :], in1=stack[0][1][:, :], op=mybir.AluOpType.add)

                        rcp = wpool.tile([P, 2 * W], f32, tag="w0")
                        scr = wpool.tile([P, 2 * W], f32, tag="w1")
                        nc.vector.reciprocal_approx_accurate(rcp[:, :], den[:, :], scr[:, :])
                        nc.vector.tensor_tensor(out=acc[:, :], in0=acc[:, :], in1=rcp[:, :], op=mybir.AluOpType.mult)
                        nc.vector.tensor_tensor(out=acc3, in0=acc3, in1=cen, op=mybir.AluOpType.add)
                        nc.sync.dma_start(out=y[ds(gbase + r0, P), :], in_=acc[:, 0:W])
                        nc.sync.dma_start(out=y[ds(gbase + r0 + P, P), :], in_=acc[:, W : 2 * W])
                    continue_images = True  # marker; fp32 path below skipped
                for b in range(n_bands if variant == "fp32" else 0):
                    r0 = b * P
                    T = []
                    for dy in range(K):
                        t_dy = bpool.tile([P, Wp], f32, tag=f"T{dy}")
                        nc.sync.dma_start(out=t_dy[:, :], in_=pad[r0 + dy : r0 + dy + P, :])
                        T.append(t_dy)
                    cen = T[R][:, R : R + W]

                    if variant == "fp32":
                        num = apool.tile([P, W], f32, tag="num")
                        den = apool.tile([P, W], f32, tag="den")
                        nc.scalar.copy(num[:, :], cen)
                        nc.gpsimd.memset(den[:, :], 1.0)

                        for dy in range(K):
                            for dx in range(K):
                                if dy == R and dx == R:
                                    continue
                                s = T[dy][:, dx : dx + W]
                                q = wpool.tile([P, W], f32, tag="q")
                                w = wpool.tile([P, W], f32, tag="w")
                                t = wpool.tile([P, W], f32, tag="t")
                                nc.vector.tensor_tensor(out=q[:, :], in0=s, in1=cen, op=mybir.AluOpType.subtract)
                                nc.vector.tensor_tensor(out=q[:, :], in0=q[:, :], in1=q[:, :], op=mybir.AluOpType.mult)
                                ds2 = (dy - R) ** 2 + (dx - R) ** 2
                                nc.scalar.activation(
                                    w[:, :],
                                    q[:, :],
                                    mybir.ActivationFunctionType.Exp,
                                    bias=bias[:, ds2_col[ds2] : ds2_col[ds2] + 1],
                                    scale=-INV_2C2,
                                )
                                nc.vector.tensor_tensor(out=t[:, :], in0=w[:, :], in1=s, op=mybir.AluOpType.mult)
                                nc.vector.tensor_tensor(out=num[:, :], in0=num[:, :], in1=t[:, :], op=mybir.AluOpType.add)
                                nc.gpsimd.tensor_tensor(out=den[:, :], in0=den[:, :], in1=w[:, :], op=mybir.AluOpType.add)

                        rcp = wpool.tile([P, W], f32, tag="rcp")
                        scr = wpool.tile([P, W], f32, tag="scr")
                        nc.vector.reciprocal_approx_accurate(rcp[:, :], den[:, :], scr[:, :])
                        nc.vector.tensor_tensor(out=num[:, :], in0=num[:, :], in1=rcp[:, :], op=mybir.AluOpType.mult)
                        nc.sync.dma_start(out=y[ds(gbase + r0, P), :], in_=num[:, :])
                    else:  # bf16A
                        acc = apool.tile([P, W], f32, tag="acc")
                        den = apool.tile([P, W], f32, tag="den")
                        nc.vector.memset(acc[:, :], 0.0)
                        nc.gpsimd.memset(den[:, :], 1.0)

                        taps = [(dy, dx) for dy in range(K) for dx in range(K) if not (dy == R and dx == R)]
                        GRP = 8  # taps per bf16 partial-sum tree
                        sub_flip = 0

                        def fold_push(stack, tile, eng):
                            # binary-counter balanced fold: stack holds (level, tile)
                            lv = 0
                            while stack and stack[-1][0] == lv:
                                _, prev = stack.pop()
                                eng.tensor_tensor(out=prev[:, :], in0=prev[:, :], in1=tile[:, :], op=mybir.AluOpType.add)
                                tile = prev
                                lv += 1
                            stack.append((lv, tile))

                        for g0 in range(0, len(taps), GRP):
                            group = taps[g0 : g0 + GRP]
                            tstack, wstack = [], []
                            for gi, (dy, dx) in enumerate(group):
                                s = T[dy][:, dx : dx + W]
                                d = wpool.tile([P, W], bf16, tag=f"d{gi % 2}")
                                q = wpool.tile([P, W], bf16, tag="q")
                                w = wpool.tile([P, W], bf16, tag=f"w{gi % 4}")
                                t = wpool.tile([P, W], bf16, tag=f"t{gi % 4}")
                                # d = s - cen  (fp32 in, bf16 out); 1/3 of subs on gpsimd
                                sub_eng = nc.gpsimd if (sub_flip % 3 == 2) else nc.vector
                                sub_flip += 1
                                sub_eng.tensor_tensor(out=d[:, :], in0=s, in1=cen, op=mybir.AluOpType.subtract)
                                nc.vector.tensor_tensor(out=q[:, :], in0=d[:, :], in1=d[:, :], op=mybir.AluOpType.mult)
                                ds2 = (dy - R) ** 2 + (dx - R) ** 2
                                nc.scalar.activation(
                                    w[:, :],
                                    q[:, :],
                                    mybir.ActivationFunctionType.Exp,
                                    bias=bias[:, ds2_col[ds2] : ds2_col[ds2] + 1],
                                    scale=-INV_2C2,
                                )
                                nc.vector.tensor_tensor(out=t[:, :], in0=w[:, :], in1=d[:, :], op=mybir.AluOpType.mult)
                                fold_push(tstack, t, nc.vector)
                                fold_push(wstack, w, nc.gpsimd)
                            # fold leftovers, then fp32 root add
                            for stack, accum, eng in ((tstack, acc, nc.vector), (wstack, den, nc.gpsimd)):
                                while len(stack) > 1:
                                    _, b2 = stack.pop()
                                    _, a2 = stack.pop()
                                    eng.tensor_tensor(out=a2[:, :], in0=a2[:, :], in1=b2[:, :], op=mybir.AluOpType.add)
                                    stack.append((99, a2))
                                eng.tensor_tensor(out=accum[:, :], in0=accum[:, :], in1=stack[0][1][:, :], op=mybir.AluOpType.add)

                        rcp = wpool.tile([P, W], f32, tag="rcp")
                        scr = wpool.tile([P, W], f32, tag="scr")
                        nc.vector.reciprocal_approx_accurate(rcp[:, :], den[:, :], scr[:, :])
                        nc.vector.tensor_tensor(out=acc[:, :], in0=acc[:, :], in1=rcp[:, :], op=mybir.AluOpType.mult)
                        nc.vector.tensor_tensor(out=acc[:, :], in0=acc[:, :], in1=cen, op=mybir.AluOpType.add)
                        nc.sync.dma_start(out=y[ds(gbase + r0, P), :], in_=acc[:, :])

    nc.finalize()
    return nc


def _get_nc(n_img, H, W, variant="fp16A"):
    key = (n_img, H, W, variant)
    if key not in _NC_CACHE:
        _NC_CACHE[key] = build_nc(n_img, H, W, variant)
    return _NC_CACHE[key]


def run_sharded(flat, n_img_per_core, H, W, trace=False, variant="fp16A"):
    """flat: [N_CORES * n_img_per_core, H, W] fp32. Returns same-shape output
    (and the BassKernelResults when trace)."""
    from concourse.bass_utils import run_bass_kernel_spmd

    nc = _get_nc(n_img_per_core, H, W, variant)
    in_maps = [
        {
            "x": np.ascontiguousarray(
                flat[c * n_img_per_core : (c + 1) * n_img_per_core].reshape(n_img_per_core * H, W)
            )
        }
        for c in range(N_CORES)
    ]
    res = run_bass_kernel_spmd(nc, in_maps, core_ids=list(range(N_CORES)), trace=trace)
    out = np.stack([res.results[c]["y"].reshape(n_img_per_core, H, W) for c in range(N_CORES)])
    return out.reshape(N_CORES * n_img_per_core, H, W), res


def kernel(input_tensor: np.ndarray) -> np.ndarray:
    input_tensor = np.asarray(input_tensor, dtype=np.float32)
    B, C, H, W = input_tensor.shape
    flat = input_tensor.reshape(B * C, H, W)
    assert (B * C) % N_CORES == 0
    out, _ = run_sharded(flat, (B * C) // N_CORES, H, W)
    return out.reshape(B, C, H, W)



# revision 3
# speedup vs baseline: 2.0370x; 2.0370x over previous
"""Bilateral filter (7x7, sigma_color=0.1) Trainium2 Bass kernel.

Full inputs: input_tensor [16, 3, 1024, 1024] fp32 in [0,1].
Sharding: batch-parallel — 48 channel-images split as 6 per core across 8 cores.

v2 "mm" design (per core, For_i over its 6 channel-images):
  Phase 0: cast the image to fp16 once, building an edge-padded copy
    pad16 [H+6, W+8] in a DRAM pool tile (rows -3..H+2, cols -4..W+3).
  Per 256-row band-pair (2 bands of 128 rows side by side in the free dim):
    - DMA 14 row/parity-shifted tiles T[dy]/To[dy] [128, 2*1030] fp16.
      Odd-parity copies make every DVE sub 4B-aligned -> 2x perf mode.
    - Per tap (dy,dx) != center:
        d = s - cen            (DVE fp16 2x)
        w = Derivative_Erf(sqrt(50)*d) = (2/sqrt(pi))*exp(-50 d^2)  (ACT, 1 op)
        t = w * d              (DVE fp16 2x)
        accP += k_c * t, denP += k_c * w   (TensorE scaled-identity matmuls
          into PSUM, 4 chunks of 512 each; k_c = (sqrt(pi)/2)*exp(-ds2/18)
          folds the spatial weight + the derf normalization)
      den's +1 (center tap) comes from one ones-matmul at group start.
    - Epilogue: rcp = reciprocal_approx(denP); out = cen + accP * rcp; DMA out.
  TensorE replaces all accumulation-tree adds; ACT does 1 op/tap instead of
  2 (+casts). GpSimd stays idle (SBUF contention slows DVE).
"""

import sys

sys.path.insert(0, "/opt/trn_rl_repo")

import math
import numpy as np

SPATIAL_RADIUS = 3
COLOR_RADIUS = 0.1
INV_2C2 = 1.0 / (2.0 * COLOR_RADIUS**2)  # 50.0
INV_2R2 = 1.0 / (2.0 * float(SPATIAL_RADIUS) ** 2)  # 1/18

N_CORES = 8
_NC_CACHE = {}


def build_nc(n_img, H, W):
    import concourse.bacc as bacc
    import concourse.bass as bass
    import concourse.mybir as mybir
    from concourse.tile import TileContext

    ds = bass.ds
    f32 = mybir.dt.float32
    f16 = mybir.dt.float16
    R = SPATIAL_RADIUS  # 3
    K = 2 * R + 1  # 7
    P = 128
    Wt = W + 6  # T-tile width (real cols -3..W+2)
    Wp = W + 8  # pad16 width  (real cols -4..W+3)
    assert H % (2 * P) == 0
    n_bp = H // (2 * P)  # band pairs per image
    CH = 512  # matmul chunk (PSUM bank)
    NCH = 2 * W // CH  # 4 chunks per band-pair

    # spatial-weight classes: ds2 = (dy-R)^2 + (dx-R)^2
    taps = [(dy, dx) for dy in range(K) for dx in range(K) if not (dy == R and dx == R)]
    ds2_of = lambda dy, dx: (dy - R) ** 2 + (dx - R) ** 2
    classes = sorted({ds2_of(dy, dx) for dy, dx in taps})
    cls_idx = {v: i for i, v in enumerate(classes)}
    # class-major tap order (amortizes PE stationary reloads)
    taps.sort(key=lambda t: (cls_idx[ds2_of(*t)], t))
    kval = {v: (math.sqrt(math.pi) / 2.0) * math.exp(-v * INV_2R2) for v in classes}

    nc = bacc.Bacc(None, target_bir_lowering=False)
    x = nc.declare_dram_parameter("x", [n_img * H, W], f32, isOutput=False)
    y = nc.declare_dram_parameter("y", [n_img * H, W], f32, isOutput=True)

    with TileContext(nc) as tc:
        with (
            tc.tile_pool(name="consts", bufs=1) as cpool,
            tc.tile_pool(name="drampool", bufs=2, space="DRAM") as dpool,
            tc.tile_pool(name="prepool", bufs=2) as ppool,
            tc.tile_pool(name="bandpool", bufs=2) as bpool,
            tc.tile_pool(name="workpool", bufs=1) as wpool,
            tc.tile_pool(name="psumpool", bufs=1, space="PSUM") as pspool,
        ):
            # ---- constants: scaled identities (one per ds2 class) + ones ----
            idents = {}
            for v in classes:
                t_id = cpool.tile([P, P], f16, tag=f"id{v}")
                nc.gpsimd.memset(t_id[:, :], 0.0)
                nc.gpsimd.affine_select(
                    out=t_id[:, :], in_=t_id[:, :],
                    compare_op=mybir.AluOpType.not_equal,
                    fill=kval[v], base=0, pattern=[[-1, P]], channel_multiplier=1,
                )
                idents[v] = t_id
            ones_lhs = cpool.tile([P, P], f16, tag="ones_lhs")
            nc.gpsimd.memset(ones_lhs[:, :], 1.0 / P)
            ones_rhs = cpool.tile([P, CH], f16, tag="ones_rhs")
            nc.gpsimd.memset(ones_rhs[:, :], 1.0)

            with tc.For_i(0, n_img * H, H) as gbase:
                # ---- phase 0: fp16 edge-padded image in DRAM ----
                pad = dpool.tile([H + 2 * R, Wp], f16, tag="pad")
                for k in range(H // P):
                    xb = ppool.tile([P, W], f32, tag="xb")
                    pb = ppool.tile([P, W], f16, tag="pb")
                    nc.sync.dma_start(out=xb[:, :], in_=x[ds(gbase + k * P, P), :])
                    nc.scalar.copy(pb[:, :], xb[:, :])
                    nc.sync.dma_start(out=pad[R + k * P : R + (k + 1) * P, 4 : 4 + W], in_=pb[:, :])
                for r in range(R):
                    nc.sync.dma_start(out=pad[r : r + 1, 4 : 4 + W], in_=pad[R : R + 1, 4 : 4 + W])
                    nc.sync.dma_start(
                        out=pad[H + R + r : H + R + r + 1, 4 : 4 + W],
                        in_=pad[H + R - 1 : H + R, 4 : 4 + W],
                    )
                with nc.allow_non_contiguous_dma(reason="edge-column pads"):
                    for c in range(4):
                        nc.sync.dma_start(out=pad[:, c : c + 1], in_=pad[:, 4:5])
                    for c in range(W + 4, Wp):
                        nc.sync.dma_start(out=pad[:, c : c + 1], in_=pad[:, W + 3 : W + 4])

                # ---- band pairs ----
                for pb_i in range(n_bp):
                    r0 = pb_i * 2 * P

                    # load 14 parity/row-shifted tiles [128, 2, Wt]
                    T = {}
                    for dy in range(K):
                        for par in range(2):  # 0: cols -3.., 1: cols -2..
                            tt = bpool.tile([P, 2 * Wt], f16, tag=f"T{dy}p{par}")
                            cc0 = 1 + par
                            for b in range(2):
                                rr = r0 + b * P + dy  # pad row of partition 0
                                nc.sync.dma_start(
                                    out=tt[:, b * Wt : (b + 1) * Wt],
                                    in_=pad[rr : rr + P, cc0 : cc0 + Wt],
                                )
                            T[(dy, par)] = tt

                    def seg(tile_, off):
                        return tile_[:, :].rearrange("p (s c) -> p s c", c=Wt)[:, :, off : off + W]

                    cen = seg(T[(R, 1)], 2)  # xp(row, c) at even offset

                    accP = pspool.tile([P, 2 * W], f32, tag="acc")
                    denP = pspool.tile([P, 2 * W], f32, tag="den")

                    # den = 1 (center tap): ones-matmul opens each den chunk
                    for j in range(NCH):
                        nc.tensor.matmul(
                            denP[:, j * CH : (j + 1) * CH], ones_lhs[:, :], ones_rhs[:, :],
                            start=True, stop=False,
                        )

                    for ti, (dy, dx) in enumerate(taps):
                        o = dx  # = delta_x + 3, in 0..6
                        if o % 2 == 0:
                            s_ap = seg(T[(dy, 0)], o)
                        else:
                            s_ap = seg(T[(dy, 1)], o - 1)
                        d = wpool.tile([P, 2 * W], f16, tag=f"d{ti % 3}")
                        w = wpool.tile([P, 2 * W], f16, tag=f"w{ti % 3}")
                        t = wpool.tile([P, 2 * W], f16, tag=f"t{ti % 3}")
                        d3 = d[:, :].rearrange("p (s c) -> p s c", c=W)
                        nc.vector.tensor_tensor(out=d3, in0=s_ap, in1=cen, op=mybir.AluOpType.subtract)
                        nc.scalar.activation(
                            w[:, :], d[:, :],
                            mybir.ActivationFunctionType.Derivative_Erf,
                            scale=math.sqrt(INV_2C2),
                        )
                        nc.vector.tensor_tensor(out=t[:, :], in0=w[:, :], in1=d[:, :], op=mybir.AluOpType.mult)
                        lhs = idents[ds2_of(dy, dx)]
                        first = ti == 0
                        last = ti == len(taps) - 1
                        for j in range(NCH):
                            nc.tensor.matmul(
                                accP[:, j * CH : (j + 1) * CH], lhs[:, :], t[:, j * CH : (j + 1) * CH],
                                start=first, stop=last,
                            )
                            nc.tensor.matmul(
                                denP[:, j * CH : (j + 1) * CH], lhs[:, :], w[:, j * CH : (j + 1) * CH],
                                start=False, stop=last,
                            )

                    # ---- epilogue: out = cen + acc / den ----
                    rcp = wpool.tile([P, 2 * W], f32, tag="rcp")
                    scr = wpool.tile([P, 2 * W], f32, tag="scr")
                    nc.vector.reciprocal_approx_accurate(rcp[:, :], denP[:, :], scr[:, :])
                    nc.vector.tensor_tensor(out=scr[:, :], in0=accP[:, :], in1=rcp[:, :], op=mybir.AluOpType.mult)
                    scr3 = scr[:, :].rearrange("p (s c) -> p s c", c=W)
                    nc.vector.tensor_tensor(out=scr3, in0=scr3, in1=cen, op=mybir.AluOpType.add)
                    nc.sync.dma_start(out=y[ds(gbase + r0, P), :], in_=scr[:, 0:W])
                    nc.sync.dma_start(out=y[ds(gbase + r0 + P, P), :], in_=scr[:, W : 2 * W])

    nc.finalize()
    return nc


def _get_nc(n_img, H, W):
    key = (n_img, H, W)
    if key not in _NC_CACHE:
        _NC_CACHE[key] = build_nc(n_img, H, W)
    return _NC_CACHE[key]


def run_sharded(flat, n_img_per_core, H, W, trace=False):
    """flat: [N_CORES * n_img_per_core, H, W] fp32. Returns same-shape output
    (and the BassKernelResults when trace)."""
    from concourse.bass_utils import run_bass_kernel_spmd

    nc = _get_nc(n_img_per_core, H, W)
    in_maps = [
        {
            "x": np.ascontiguousarray(
                flat[c * n_img_per_core : (c + 1) * n_img_per_core].reshape(n_img_per_core * H, W)
            )
        }
        for c in range(N_CORES)
    ]
    res = run_bass_kernel_spmd(nc, in_maps, core_ids=list(range(N_CORES)), trace=trace)
    out = np.stack([res.results[c]["y"].reshape(n_img_per_core, H, W) for c in range(N_CORES)])
    return out.reshape(N_CORES * n_img_per_core, H, W), res


def kernel(input_tensor: np.ndarray) -> np.ndarray:
    input_tensor = np.asarray(input_tensor, dtype=np.float32)
    B, C, H, W = input_tensor.shape
    flat = input_tensor.reshape(B * C, H, W)
    assert (B * C) % N_CORES == 0
    out, _ = run_sharded(flat, (B * C) // N_CORES, H, W)
    return out.reshape(B, C, H, W)


# revision 5
# speedup vs baseline: 2.0806x; 1.0214x over previous
"""Bilateral filter (7x7, sigma_color=0.1) Trainium2 Bass kernel.

Full inputs: input_tensor [16, 3, 1024, 1024] fp32 in [0,1].
Sharding: batch-parallel — 48 channel-images split as 6 per core across 8 cores.

v2 "mm" design (per core, For_i over its 6 channel-images):
  Phase 0: cast the image to fp16 once, building an edge-padded copy
    pad16 [H+6, W+8] in a DRAM pool tile (rows -3..H+2, cols -4..W+3).
  Per 256-row band-pair (2 bands of 128 rows side by side in the free dim):
    - DMA 14 row/parity-shifted tiles T[dy]/To[dy] [128, 2*1030] fp16.
      Odd-parity copies make every DVE sub 4B-aligned -> 2x perf mode.
    - Per tap (dy,dx) != center:
        d = s - cen            (DVE fp16 2x)
        w = Derivative_Erf(sqrt(50)*d) = (2/sqrt(pi))*exp(-50 d^2)  (ACT, 1 op)
        t = w * d              (DVE fp16 2x)
        accP += k_c * t, denP += k_c * w   (TensorE scaled-identity matmuls
          into PSUM, 4 chunks of 512 each; k_c = (sqrt(pi)/2)*exp(-ds2/18)
          folds the spatial weight + the derf normalization)
      den's +1 (center tap) comes from one ones-matmul at group start.
    - Epilogue: rcp = reciprocal_approx(denP); out = cen + accP * rcp; DMA out.
  TensorE replaces all accumulation-tree adds; ACT does 1 op/tap instead of
  2 (+casts). GpSimd stays idle (SBUF contention slows DVE).
"""

import sys

sys.path.insert(0, "/opt/trn_rl_repo")

import math
import numpy as np

SPATIAL_RADIUS = 3
COLOR_RADIUS = 0.1
INV_2C2 = 1.0 / (2.0 * COLOR_RADIUS**2)  # 50.0
INV_2R2 = 1.0 / (2.0 * float(SPATIAL_RADIUS) ** 2)  # 1/18

N_CORES = 8
_NC_CACHE = {}


def build_nc(n_img, H, W):
    import concourse.bacc as bacc
    import concourse.bass as bass
    import concourse.mybir as mybir
    from concourse.tile import TileContext

    ds = bass.ds
    f32 = mybir.dt.float32
    f16 = mybir.dt.float16
    R = SPATIAL_RADIUS  # 3
    K = 2 * R + 1  # 7
    P = 128
    Wt = W + 6  # T-tile width (real cols -3..W+2)
    Wp = W + 8  # pad16 width  (real cols -4..W+3)
    assert H % (2 * P) == 0
    n_bp = H // (2 * P)  # band pairs per image
    CH = 512  # matmul chunk (PSUM bank)
    NCH = 2 * W // CH  # 4 chunks per band-pair

    # spatial-weight classes: ds2 = (dy-R)^2 + (dx-R)^2
    taps = [(dy, dx) for dy in range(K) for dx in range(K) if not (dy == R and dx == R)]
    ds2_of = lambda dy, dx: (dy - R) ** 2 + (dx - R) ** 2
    classes = sorted({ds2_of(dy, dx) for dy, dx in taps})
    cls_idx = {v: i for i, v in enumerate(classes)}
    # class-major tap order (amortizes PE stationary reloads)
    taps.sort(key=lambda t: (cls_idx[ds2_of(*t)], t))
    kval = {v: (math.sqrt(math.pi) / 2.0) * math.exp(-v * INV_2R2) for v in classes}

    nc = bacc.Bacc(None, target_bir_lowering=False)
    x = nc.declare_dram_parameter("x", [n_img * H, W], f32, isOutput=False)
    y = nc.declare_dram_parameter("y", [n_img * H, W], f32, isOutput=True)

    with TileContext(nc) as tc:
        with (
            tc.tile_pool(name="consts", bufs=1) as cpool,
            tc.tile_pool(name="drampool", bufs=2, space="DRAM") as dpool,
            tc.tile_pool(name="prepool", bufs=2) as ppool,
            tc.tile_pool(name="bandpool", bufs=2) as bpool,
            tc.tile_pool(name="workpool", bufs=1) as wpool,
            tc.tile_pool(name="psumpool", bufs=1, space="PSUM") as pspool,
        ):
            # ---- constants: scaled identities (one per ds2 class) + ones ----
            idents = {}
            for v in classes:
                t_id = cpool.tile([P, P], f16, tag=f"id{v}")
                nc.gpsimd.memset(t_id[:, :], 0.0)
                nc.gpsimd.affine_select(
                    out=t_id[:, :], in_=t_id[:, :],
                    compare_op=mybir.AluOpType.not_equal,
                    fill=kval[v], base=0, pattern=[[-1, P]], channel_multiplier=1,
                )
                idents[v] = t_id
            ones_lhs = cpool.tile([P, P], f16, tag="ones_lhs")
            nc.gpsimd.memset(ones_lhs[:, :], 1.0 / P)
            ones_rhs = cpool.tile([P, CH], f16, tag="ones_rhs")
            nc.gpsimd.memset(ones_rhs[:, :], 1.0)

            with tc.For_i(0, n_img * H, H) as gbase:
                # ---- phase 0: fp16 edge-padded image in DRAM ----
                # Entirely on the (otherwise idle) GpSimd engine + its DMA
                # queue so it overlaps the previous image's band compute
                # (the Sync queue is head-blocking FIFO).
                pad = dpool.tile([H + 2 * R, Wp], f16, tag="pad")
                for k in range(H // P):
                    xb = ppool.tile([P, W], f32, tag="xb")
                    pb = ppool.tile([P, Wp], f16, tag="pb")
                    nc.gpsimd.dma_start(out=xb[:, :], in_=x[ds(gbase + k * P, P), :])
                    nc.gpsimd.tensor_copy(pb[:, 4 : 4 + W], xb[:, :])
                    for c in range(4):
                        nc.gpsimd.tensor_copy(pb[:, c : c + 1], pb[:, 4:5])
                    for c in range(W + 4, Wp):
                        nc.gpsimd.tensor_copy(pb[:, c : c + 1], pb[:, W + 3 : W + 4])
                    nc.gpsimd.dma_start(out=pad[R + k * P : R + (k + 1) * P, :], in_=pb[:, :])
                for r in range(R):
                    nc.gpsimd.dma_start(out=pad[r : r + 1, :], in_=pad[R : R + 1, :])
                    nc.gpsimd.dma_start(
                        out=pad[H + R + r : H + R + r + 1, :],
                        in_=pad[H + R - 1 : H + R, :],
                    )

                # ---- band pairs ----
                for pb_i in range(n_bp):
                    r0 = pb_i * 2 * P

                    # load 14 parity/row-shifted tiles [128, 2, Wt]
                    T = {}
                    for dy in range(K):
                        for par in range(2):  # 0: cols -3.., 1: cols -2..
                            tt = bpool.tile([P, 2 * Wt], f16, tag=f"T{dy}p{par}")
                            cc0 = 1 + par
                            for b in range(2):
                                rr = r0 + b * P + dy  # pad row of partition 0
                                nc.sync.dma_start(
                                    out=tt[:, b * Wt : (b + 1) * Wt],
                                    in_=pad[rr : rr + P, cc0 : cc0 + Wt],
                                )
                            T[(dy, par)] = tt

                    def seg(tile_, off):
                        return tile_[:, :].rearrange("p (s c) -> p s c", c=Wt)[:, :, off : off + W]

                    cen = seg(T[(R, 1)], 2)  # xp(row, c) at even offset

                    accP = pspool.tile([P, 2 * W], f32, tag="acc")
                    denP = pspool.tile([P, 2 * W], f32, tag="den")

                    # den = 1 (center tap): ones-matmul opens each den chunk
                    for j in range(NCH):
                        nc.tensor.matmul(
                            denP[:, j * CH : (j + 1) * CH], ones_lhs[:, :], ones_rhs[:, :],
                            start=True, stop=False,
                        )

                    for ti, (dy, dx) in enumerate(taps):
                        o = dx  # = delta_x + 3, in 0..6
                        if o % 2 == 0:
                            s_ap = seg(T[(dy, 0)], o)
                        else:
                            s_ap = seg(T[(dy, 1)], o - 1)
                        d = wpool.tile([P, 2 * W], f16, tag=f"d{ti % 3}")
                        w = wpool.tile([P, 2 * W], f16, tag=f"w{ti % 3}")
                        t = wpool.tile([P, 2 * W], f16, tag=f"t{ti % 3}")
                        d3 = d[:, :].rearrange("p (s c) -> p s c", c=W)
                        nc.vector.tensor_tensor(out=d3, in0=s_ap, in1=cen, op=mybir.AluOpType.subtract)
                        nc.scalar.activation(
                            w[:, :], d[:, :],
                            mybir.ActivationFunctionType.Derivative_Erf,
                            scale=math.sqrt(INV_2C2),
                        )
                        nc.vector.tensor_tensor(out=t[:, :], in0=w[:, :], in1=d[:, :], op=mybir.AluOpType.mult)
                        lhs = idents[ds2_of(dy, dx)]
                        first = ti == 0
                        last = ti == len(taps) - 1
                        for j in range(NCH):
                            nc.tensor.matmul(
                                accP[:, j * CH : (j + 1) * CH], lhs[:, :], t[:, j * CH : (j + 1) * CH],
                                start=first, stop=last,
                            )
                            nc.tensor.matmul(
                                denP[:, j * CH : (j + 1) * CH], lhs[:, :], w[:, j * CH : (j + 1) * CH],
                                start=False, stop=last,
                            )

                    # ---- epilogue: out = cen + acc / den ----
                    # ACT evacuates PSUM immediately (frees banks for the next
                    # band-pair's matmuls ~10us earlier than the DVE chain).
                    accS = wpool.tile([P, 2 * W], f32, tag="accS")
                    denS = wpool.tile([P, 2 * W], f32, tag="denS")
                    nc.scalar.copy(accS[:, :], accP[:, :])
                    nc.scalar.copy(denS[:, :], denP[:, :])
                    rcp = wpool.tile([P, 2 * W], f32, tag="rcp")
                    scr = wpool.tile([P, 2 * W], f32, tag="scr")
                    nc.vector.reciprocal_approx_accurate(rcp[:, :], denS[:, :], scr[:, :])
                    nc.vector.tensor_tensor(out=scr[:, :], in0=accS[:, :], in1=rcp[:, :], op=mybir.AluOpType.mult)
                    scr3 = scr[:, :].rearrange("p (s c) -> p s c", c=W)
                    nc.vector.tensor_tensor(out=scr3, in0=scr3, in1=cen, op=mybir.AluOpType.add)
                    nc.sync.dma_start(out=y[ds(gbase + r0, P), :], in_=scr[:, 0:W])
                    nc.sync.dma_start(out=y[ds(gbase + r0 + P, P), :], in_=scr[:, W : 2 * W])

    nc.finalize()
    return nc


def _get_nc(n_img, H, W):
    key = (n_img, H, W)
    if key not in _NC_CACHE:
        _NC_CACHE[key] = build_nc(n_img, H, W)
    return _NC_CACHE[key]


def run_sharded(flat, n_img_per_core, H, W, trace=False):
    """flat: [N_CORES * n_img_per_core, H, W] fp32. Returns same-shape output
    (and the BassKernelResults when trace)."""
    from concourse.bass_utils import run_bass_kernel_spmd

    nc = _get_nc(n_img_per_core, H, W)
    in_maps = [
        {
            "x": np.ascontiguousarray(
                flat[c * n_img_per_core : (c + 1) * n_img_per_core].reshape(n_img_per_core * H, W)
            )
        }
        for c in range(N_CORES)
    ]
    res = run_bass_kernel_spmd(nc, in_maps, core_ids=list(range(N_CORES)), trace=trace)
    out = np.stack([res.results[c]["y"].reshape(n_img_per_core, H, W) for c in range(N_CORES)])
    return out.reshape(N_CORES * n_img_per_core, H, W), res


def kernel(input_tensor: np.ndarray) -> np.ndarray:
    input_tensor = np.asarray(input_tensor, dtype=np.float32)
    B, C, H, W = input_tensor.shape
    flat = input_tensor.reshape(B * C, H, W)
    assert (B * C) % N_CORES == 0
    out, _ = run_sharded(flat, (B * C) // N_CORES, H, W)
    return out.reshape(B, C, H, W)


# revision 13
# speedup vs baseline: 2.9378x; 1.4120x over previous
"""Bilateral filter v3: symmetric-pair sharing + TensorE shift-matrix accumulation.

Math: for each UNORDERED tap pair {+d, -d} (24 reps), the A-form contribution
shares one computation:
    d'(q) = xp(q+delta) - xp(q)     (computed at every padded center q)
    w'(q) = (2/sqrt(pi)) exp(-50 d'^2)   [ACT Derivative_Erf]
    t'(q) = w'(q) * d'(q)
  tap +d at p:  acc += k*t'(p),        den += k*w'(p)
  tap -d at p:  acc -= k*t'(p-delta),  den += k*w'(p-delta)   [exact: d' odd, w' even]
Row shifts (p-delta_y) ride the TensorE stationary (shifted scaled diagonal),
col shifts (delta_x) ride the matmul RHS free-dim offset.

Bands: tiles cover padded rows r0-3 .. r0+124 (128 partitions); the shifted
reads make only 125 output rows/band valid -> 9 overlapped bands per image
(r0 = 0,125,...,875,899; the last re-computes rows 899..999, harmless).
Band pairs pack 2 bands in the free dim; the 9th runs alone.

DVE: 2 ops/pair (sub, mul).  ACT: 1 derf/pair + PSUM evac (evac of den adds
the center tap's +1 via the activation bias).  TensorE: 16 chunk-matmuls/pair.
Prepass/pipeline/deferred-epilogue structure as v2.2.
"""

import sys

sys.path.insert(0, "/opt/trn_rl_repo")

import math
import numpy as np

SPATIAL_RADIUS = 3
COLOR_RADIUS = 0.1
INV_2C2 = 1.0 / (2.0 * COLOR_RADIUS**2)  # 50.0
INV_2R2 = 1.0 / (2.0 * float(SPATIAL_RADIUS) ** 2)  # 1/18

N_CORES = 8
_NC_CACHE = {}


def build_nc(n_img, H, W):
    import concourse.bacc as bacc
    import concourse.bass as bass
    import concourse.mybir as mybir
    from concourse.tile import TileContext

    f32 = mybir.dt.float32
    f16 = mybir.dt.float16
    R = SPATIAL_RADIUS  # 3
    P = 128
    Wd = W + 6  # d/w/t width: centers c' = -3..W+2
    Wt = W + 12  # T-tile width: real cols -6..W+5
    Wp = W + 14  # pad16 width: real cols -6..W+7 (even, covers T-odd j<=1035)
    CH = 512
    VR = P - 3  # valid output rows per band (125)

    # band starts (output rows r0..r0+124 per band)
    starts = list(range(0, H - VR + 1, VR))
    if starts[-1] != H - VR:
        starts.append(H - VR)
    bps = [tuple(starts[i : i + 2]) for i in range(0, len(starts) - 1, 2)]
    if len(starts) % 2 == 1:
        bps.append((starts[-1],))

    # pair representatives (delta_y, delta_x)
    reps = [(0, dx) for dx in (1, 2, 3)] + [(dy, dx) for dy in (1, 2, 3) for dx in range(-3, 4)]
    ds2_of = lambda dy, dx: dy * dy + dx * dx
    classes = sorted({ds2_of(*r) for r in reps})
    cls_idx = {v: i for i, v in enumerate(classes)}
    reps.sort(key=lambda r: (cls_idx[ds2_of(*r)], r))
    kval = {v: (math.sqrt(math.pi) / 2.0) * math.exp(-v * INV_2R2) for v in classes}

    nc = bacc.Bacc(None, target_bir_lowering=False)
    x = nc.declare_dram_parameter("x", [n_img * H, W], f32, isOutput=False)
    y = nc.declare_dram_parameter("y", [n_img * H, W], f32, isOutput=True)

    with TileContext(nc) as tc:
        with (
            tc.tile_pool(name="consts", bufs=1) as cpool,
            tc.tile_pool(name="drampool", bufs=1, space="DRAM") as dpool,
            tc.tile_pool(name="prepool", bufs=2) as ppool,
            tc.tile_pool(name="bandpool", bufs=2) as bpool,
            tc.tile_pool(name="workpool", bufs=1) as wpool,
            tc.tile_pool(name="psumpool", bufs=1, space="PSUM") as pspool,
        ):
            # ---- stationaries: shifted scaled diagonals ----
            def diag(tag, off, fill):
                # nonzero at k == m + off  (k partition, m free)
                t_id = cpool.tile([P, P], f16, tag=tag)
                nc.gpsimd.memset(t_id[:, :], 0.0)
                nc.gpsimd.affine_select(
                    out=t_id[:, :], in_=t_id[:, :],
                    compare_op=mybir.AluOpType.not_equal,
                    fill=fill, base=-off, pattern=[[-1, P]], channel_multiplier=1,
                )
                return t_id

            id_plus = {v: diag(f"p{v}", 3, kval[v]) for v in classes}
            id_macc = {}
            id_mden = {}
            for v in classes:
                for dy in sorted({r[0] for r in reps if ds2_of(*r) == v}):
                    id_macc[(v, dy)] = diag(f"a{v}_{dy}", 3 - dy, -kval[v])
                    id_mden[(v, dy)] = diag(f"d{v}_{dy}", 3 - dy, kval[v])
            # delta_x == 0 pairs: both terms read the same RHS columns, so the
            # two diagonals merge into ONE stationary -> 1 matmul per target
            id_m2acc = {}
            id_m2den = {}
            for dy, dx in reps:
                if dx == 0:
                    v = ds2_of(dy, dx)
                    ta = diag(f"ma{dy}", 3, kval[v])
                    nc.gpsimd.affine_select(
                        out=ta[:, :], in_=ta[:, :],
                        compare_op=mybir.AluOpType.not_equal,
                        fill=-kval[v], base=-(3 - dy), pattern=[[-1, P]], channel_multiplier=1,
                    )
                    id_m2acc[dy] = ta
                    td = diag(f"md{dy}", 3, kval[v])
                    nc.gpsimd.affine_select(
                        out=td[:, :], in_=td[:, :],
                        compare_op=mybir.AluOpType.not_equal,
                        fill=kval[v], base=-(3 - dy), pattern=[[-1, P]], channel_multiplier=1,
                    )
                    id_m2den[dy] = td

            def emit_prepass(gb, buf, fast=False):
                """Build pad16 (rows -3..H+2, cols -6..W+7). Normally GpSimd-only
                (overlaps band compute); `fast` spreads the first image's
                prepass over the still-idle ACT/DVE engines + sync queue."""
                pad = dpool.tile([H + 2 * R, Wp], f16, tag=f"pad{buf}")
                dma = nc.sync.dma_start if fast else nc.gpsimd.dma_start
                for k in range(H // P):
                    xb = ppool.tile([P, W], f32, tag="xb")
                    pb = ppool.tile([P, Wp], f16, tag="pb")
                    dma(out=xb[:, :], in_=x[gb + k * P : gb + (k + 1) * P, :])
                    if fast:
                        nc.scalar.copy(pb[:, 6 : 6 + W], xb[:, :])
                    else:
                        nc.gpsimd.tensor_copy(pb[:, 6 : 6 + W], xb[:, :])
                    cols = list(range(6)) + list(range(W + 6, Wp))
                    for i, c in enumerate(cols):
                        src = 6 if c < 6 else W + 5
                        if fast:
                            if i % 2:
                                nc.vector.tensor_copy(pb[:, c : c + 1], pb[:, src : src + 1])
                            else:
                                nc.scalar.copy(pb[:, c : c + 1], pb[:, src : src + 1])
                        else:
                            nc.gpsimd.tensor_copy(pb[:, c : c + 1], pb[:, src : src + 1])
                    dma(out=pad[R + k * P : R + (k + 1) * P, :], in_=pb[:, :])
                for r in range(R):
                    dma(out=pad[r : r + 1, :], in_=pad[R : R + 1, :])
                    dma(
                        out=pad[H + R + r : H + R + r + 1, :],
                        in_=pad[H + R - 1 : H + R, :],
                    )
                return pad

            pending_epi = []

            def emit_image(gb, pad):
                def emit_bp(segs):
                    ns = len(segs)
                    FD = ns * Wd  # d/w/t free dim
                    FO = ns * W  # acc/den/out free dim

                    # T tiles: dy' = delta_y in 0..3, two parities
                    # T_even[dy][k, b, j] = xp(r0b-3+k+dy, j-6); T_odd: j-5
                    T = {}
                    for dy in range(4):
                        for par in range(2):
                            tt = bpool.tile([P, ns * Wt], f16, tag=f"T{dy}p{par}")
                            for b, r0 in enumerate(segs):
                                nc.sync.dma_start(
                                    out=tt[:, b * Wt : (b + 1) * Wt],
                                    in_=pad[r0 + dy : r0 + dy + P, par : par + Wt],
                                )
                            T[(dy, par)] = tt
                    # center rows for the final add: cen_epi[m, b, c] = xp(r0b+m, c)
                    cen_epi = bpool.tile([P, FO], f16, tag=f"cen_epi{ns}")
                    for b, r0 in enumerate(segs):
                        nc.sync.dma_start(
                            out=cen_epi[:, b * W : (b + 1) * W],
                            in_=pad[r0 + 3 : r0 + 3 + P, 6 : 6 + W],
                        )

                    def tseg(tile_, off):
                        return tile_[:, :].rearrange("p (s c) -> p s c", c=Wt)[:, :, off : off + Wd]

                    cen = tseg(T[(0, 1)], 2)  # xp(q, c') at even offset

                    # always full-size (4 banks each) so the single-band bp
                    # reuses the same 8 PSUM banks instead of new ones
                    accP = pspool.tile([P, 2 * W], f32, tag="acc")
                    denP = pspool.tile([P, 2 * W], f32, tag="den")

                    for ri, (dy, dx) in enumerate(reps):
                        o = dx + 3
                        if o % 2 == 0:
                            s_ap = tseg(T[(dy, 0)], o)
                        else:
                            s_ap = tseg(T[(dy, 1)], o - 1)
                        d = wpool.tile([P, FD], f16, tag=f"d{ri % 3}_{ns}")
                        w = wpool.tile([P, FD], f16, tag=f"w{ri % 3}_{ns}")
                        t = wpool.tile([P, FD], f16, tag=f"t{ri % 3}_{ns}")
                        d3 = d[:, :].rearrange("p (s c) -> p s c", c=Wd)
                        nc.vector.tensor_tensor(out=d3, in0=s_ap, in1=cen, op=mybir.AluOpType.subtract)
                        nc.scalar.activation(
                            w[:, :], d[:, :],
                            mybir.ActivationFunctionType.Derivative_Erf,
                            scale=math.sqrt(INV_2C2),
                        )
                        nc.vector.tensor_tensor(out=t[:, :], in0=w[:, :], in1=d[:, :], op=mybir.AluOpType.mult)

                        v = ds2_of(dy, dx)
                        first = ri == 0
                        last = ri == len(reps) - 1
                        chunks = [(b * W + j * CH, b * Wd + j * CH) for b in range(ns) for j in range(W // CH)]
                        if dx == 0:
                            # merged two-diagonal stationaries: 1 matmul/target
                            ma, md = id_m2acc[dy], id_m2den[dy]
                            for oc, ub in chunks:
                                up = ub + 3
                                nc.tensor.matmul(
                                    accP[:, oc : oc + CH], ma[:, :], t[:, up : up + CH],
                                    start=first, stop=last,
                                )
                                nc.tensor.matmul(
                                    denP[:, oc : oc + CH], md[:, :], w[:, up : up + CH],
                                    start=first, stop=last,
                                )
                        else:
                            lp, la, ld = id_plus[v], id_macc[(v, dy)], id_mden[(v, dy)]
                            # group by stationary: lp (8 mms), la (4), ld (4)
                            for oc, ub in chunks:
                                up = ub + 3
                                nc.tensor.matmul(
                                    accP[:, oc : oc + CH], lp[:, :], t[:, up : up + CH],
                                    start=first, stop=False,
                                )
                                nc.tensor.matmul(
                                    denP[:, oc : oc + CH], lp[:, :], w[:, up : up + CH],
                                    start=first, stop=False,
                                )
                            for oc, ub in chunks:
                                um = ub + 3 - dx
                                nc.tensor.matmul(
                                    accP[:, oc : oc + CH], la[:, :], t[:, um : um + CH],
                                    start=False, stop=last,
                                )
                            for oc, ub in chunks:
                                um = ub + 3 - dx
                                nc.tensor.matmul(
                                    denP[:, oc : oc + CH], ld[:, :], w[:, um : um + CH],
                                    start=False, stop=last,
                                )
                        if ri == 2 and pending_epi:
                            pending_epi.pop()()

                    def epilogue(accP=accP, denP=denP, cen_epi=cen_epi, segs=segs, FO=FO):
                        accS = wpool.tile([P, FO], f32, tag=f"accS{len(segs)}")
                        denS = wpool.tile([P, FO], f32, tag=f"denS{len(segs)}")
                        nc.scalar.copy(accS[:, :], accP[:, 0:FO])
                        # den evac + the center tap's +1, fused into the bias
                        nc.scalar.activation(
                            denS[:, :], denP[:, 0:FO],
                            mybir.ActivationFunctionType.Copy, bias=1.0,
                        )
                        rcp = wpool.tile([P, FO], f32, tag=f"rcp{len(segs)}")
                        scr = wpool.tile([P, FO], f32, tag=f"scr{len(segs)}")
                        nc.vector.reciprocal_approx_accurate(rcp[:, :], denS[:, :], scr[:, :])
                        nc.vector.tensor_tensor(out=scr[:, :], in0=accS[:, :], in1=rcp[:, :], op=mybir.AluOpType.mult)
                        nc.vector.tensor_tensor(out=scr[:, :], in0=scr[:, :], in1=cen_epi[:, :], op=mybir.AluOpType.add)
                        for b, r0 in enumerate(segs):
                            nc.sync.dma_start(
                                out=y[gb + r0 : gb + r0 + VR, :],
                                in_=scr[0:VR, b * W : (b + 1) * W],
                            )

                    pending_epi.append(epilogue)

                for segs in bps:
                    emit_bp(segs)

            pads = {0: emit_prepass(0, "A", fast=True)}
            if n_img > 1:
                pads[1] = emit_prepass(H, "B")
            for i in range(n_img):
                emit_image(i * H, pads.pop(i))
                nxt = i + 2
                if nxt < n_img:
                    pads[nxt] = emit_prepass(nxt * H, "AB"[nxt % 2])
            while pending_epi:
                pending_epi.pop()()

    nc.finalize()
    return nc


def _get_nc(n_img, H, W):
    key = (n_img, H, W)
    if key not in _NC_CACHE:
        _NC_CACHE[key] = build_nc(n_img, H, W)
    return _NC_CACHE[key]


def run_sharded(flat, n_img_per_core, H, W, trace=False):
    from concourse.bass_utils import run_bass_kernel_spmd

    nc = _get_nc(n_img_per_core, H, W)
    in_maps = [
        {
            "x": np.ascontiguousarray(
                flat[c * n_img_per_core : (c + 1) * n_img_per_core].reshape(n_img_per_core * H, W)
            )
        }
        for c in range(N_CORES)
    ]
    res = run_bass_kernel_spmd(nc, in_maps, core_ids=list(range(N_CORES)), trace=trace)
    out = np.stack([res.results[c]["y"].reshape(n_img_per_core, H, W) for c in range(N_CORES)])
    return out.reshape(N_CORES * n_img_per_core, H, W), res


def kernel(input_tensor: np.ndarray) -> np.ndarray:
    input_tensor = np.asarray(input_tensor, dtype=np.float32)
    B, C, H, W = input_tensor.shape
    flat = input_tensor.reshape(B * C, H, W)
    assert (B * C) % N_CORES == 0
    out, _ = run_sharded(flat, (B * C) // N_CORES, H, W)
    return out.reshape(B, C, H, W)
